# revision 8
# baseline (speedup 1.0000x reference)
"""MoE top-1 routing kernel for Trainium2 (8 NeuronCores).

Reference computation (B=8, S=1024, D=768, E=8, F=3072):
    gates = softmax(x @ gate_w + gate_b); expert_idx = argmax(gates)
    out[t] = gelu(x[t] @ w1[e] + b1[e]) @ w2[e] + b2[e]   for e = expert_idx[t]
    (no gate-probability scaling)

Strategy:
  * Routing on host in fp64 (softmax is monotonic, so argmax of logits ==
    argmax of gates; observed top-2 logit gaps are far above fp32 noise).
  * Every core holds a 1/4 slice of the F dimension of ALL 8 experts'
    weights and processes HALF the tokens: each expert's token block is
    split ceil/floor between core-rows {0-3} and {4-7}; core q in a row
    owns features [q*768, (q+1)*768).  Both rows see identical block
    capacities ceil(count/2) (odd blocks pad one dummy token), so one
    SPMD program serves all 8 cores with at most 8 tokens of padding
    total -- essentially perfect load balance.  The host adds the four
    F-slice partials per row + b2 and scatters rows back together.
  * Matmuls in bf16 with fp32 PSUM accumulation; activations stay
    transposed ([feature, token]).  gelu (erf-based) on the Scalar
    engine with the b1 bias fused; FFN2 partial-sums are copied
    PSUM->SBUF as bf16 on the Vector engine and DMA'd out.
  * DMA plan: the Scalar queue issues NO transfers (so the activation
    table load and gelus are never queued behind descriptor setup --
    DMA issue cost scales with descriptor rows).  Expert weights stream
    just-in-time through 3-deep ring buffers as single contiguous-row
    DMAs on GpSimd; token x arrives block-pair-granular on Sync; output
    tiles leave as cheap 2D per-feature-chunk DMAs on Sync/GpSimd.
"""

import sys

try:
    import concourse  # noqa: F401
except ImportError:
    sys.path.insert(0, "/opt/trn_rl_repo")

import numpy as np
import ml_dtypes

import concourse.bass as bass  # noqa: F401
import concourse.tile as tile
import concourse.mybir as mybir
from concourse import bacc
from concourse import bass_utils

BF16 = mybir.dt.bfloat16
F32 = mybir.dt.float32
AF = mybir.ActivationFunctionType

B, S, D, E = 8, 1024, 768, 8
F = 4 * D           # 3072
T = B * S           # 8192
KD = D // 128       # 6 contraction chunks over D
FQ = F // 4         # 768 features per core (1/4 slice)
KQ = FQ // 128      # 6 chunks over the F-slice
N_CORES = 8
MAX_N = 512         # moving-dim tile (one fp32 PSUM bank)
N_WARM = 40         # PE-clock warmup matmuls

TRACE = False
LAST_RESULT = None


def _split_tiles(cap, lead=None, tail=None):
    """Split a block of `cap` tokens into near-equal tiles of <= MAX_N.
    `lead`/`tail` carve a small first/last tile (so the first matmuls
    depend on a sliver of DMA and the final output drain is short)."""
    if cap == 0:
        return []
    out = []
    off = 0
    tail_t = None
    if lead is not None and cap > lead + 128:
        out.append((0, lead))
        off = lead
        cap -= lead
    if tail is not None and cap > tail + 128:
        tail_t = tail
        cap -= tail
    n = -(-cap // MAX_N)
    base, rem = divmod(cap, n)
    for i in range(n):
        sz = base + (1 if i < rem else 0)
        out.append((off, sz))
        off += sz
    if tail_t is not None:
        out.append((off, tail_t))
    return out


def build_program(caps):
    """Per-core program: 8 expert blocks with capacities `caps` (one half
    of each expert's tokens), F/4 feature slice of every expert."""
    caps = list(caps)
    CT = sum(caps)
    nc = bacc.Bacc("TRN2", target_bir_lowering=False, debug=False,
                   num_devices=N_CORES)

    xT_d = nc.dram_tensor("xT", (128, KD, CT), BF16, kind="ExternalInput")
    w1_d = nc.dram_tensor("w1", (128, E, KQ, KD, 128), BF16,
                          kind="ExternalInput")
    w2_d = nc.dram_tensor("w2", (128, E, KD, KQ, 128), BF16,
                          kind="ExternalInput")
    b1_d = nc.dram_tensor("b1", (128, E, KQ), F32, kind="ExternalInput")
    yT_d = nc.dram_tensor("yT", (128, KD, CT), BF16, kind="ExternalOutput")

    offs = np.concatenate([[0], np.cumsum(caps)]).astype(int)
    nz = [b for b in range(E) if caps[b] > 0]
    first_b, last_b = nz[0], nz[-1]

    # (block, tile-offset, width) in execution order.
    sched = []
    for b in nz:
        lead = 256 if b == first_b else None
        tail = 128 if b == last_b else None
        for (o, w) in _split_tiles(caps[b], lead=lead, tail=tail):
            sched.append((b, offs[b] + o, w))

    with tile.TileContext(nc) as tc:
        with (
            tc.tile_pool(name="wts", bufs=1) as wts,
            tc.tile_pool(name="w2p", bufs=3) as w2p,
            tc.tile_pool(name="act", bufs=2) as actp,
            tc.tile_pool(name="yp", bufs=2) as yp,
            tc.tile_pool(name="ps1", bufs=4, space="PSUM") as ps1,
            tc.tile_pool(name="ps2", bufs=4, space="PSUM") as ps2,
        ):
            xT = wts.tile([128, KD, CT], BF16, tag="xT")
            w1 = wts.tile([128, E, KQ, KD, 128], BF16, tag="w1")
            b1 = wts.tile([128, E, KQ], F32, tag="b1")
            warm = wts.tile([128, 128], BF16, tag="warm")
            # memset on the (otherwise idle) Vector engine so GpSimd's first
            # instruction is the critical w1[e0] DMA issue.
            nc.vector.memset(warm[:], 0.0)
            wps = ps1.tile([128, 128], F32, tag="ps1",
                           padded_shape=[128, MAX_N])

            # PE warmup: dummy matmuls flip the HAM clock gate to 2.4 GHz
            # while the head DMAs stream in.
            for _ in range(N_WARM):
                nc.tensor.matmul(wps[:, :], warm[:, :], warm[:, :])

            # w2 is ring-streamed (md-pair pieces land ~9us after issue);
            # w1 is fully resident, preloaded in pieces sized to each
            # block's deadline (block i starts ~15us*i into the stream).
            w2t = {}

            def fetch_w2(b, q=None):
                w2t[b] = w2p.tile([128, KD, KQ, 128], BF16, tag="w2e",
                                  name=f"w2e{b}")
                for md in range(0, KD, 2):
                    (q or nc.gpsimd).dma_start(w2t[b][:, md:md + 2],
                                               w2_d[:, b, md:md + 2])

            # --- Head DMAs ---
            e0 = first_b
            o0, w0 = sched[0][1], sched[0][2]
            # GpSimd: w1[e0] per-m, b1, w1[e1] in thirds, w2[e1] pieces,
            # w1[e2] in halves, then w1[e3..] whole.
            for m in range(KQ):
                nc.gpsimd.dma_start(w1[:, e0, m], w1_d[:, e0, m])
            nc.gpsimd.dma_start(b1[:], b1_d[:])
            if len(nz) > 1:
                e1 = nz[1]
                for m in range(0, KQ, 2):
                    nc.gpsimd.dma_start(w1[:, e1, m:m + 2],
                                        w1_d[:, e1, m:m + 2])
                fetch_w2(e1)
            if len(nz) > 2:
                e2 = nz[2]
                for m in range(0, KQ, 3):
                    nc.gpsimd.dma_start(w1[:, e2, m:m + 3],
                                        w1_d[:, e2, m:m + 3])
            for b in nz[3:]:
                nc.gpsimd.dma_start(w1[:, b], w1_d[:, b])
            # Sync: first tile's tokens (per k-pair), then the rest of the
            # first block, then w2[e0] md-pairs, then x per block-pair.
            for k in range(0, KD, 2):
                nc.sync.dma_start(xT[:, k:k + 2, o0:o0 + w0],
                                  xT_d[:, k:k + 2, o0:o0 + w0])
            if caps[e0] > w0:
                a, z = offs[e0] + w0, offs[e0 + 1]
                for k in range(0, KD, 2):
                    nc.sync.dma_start(xT[:, k:k + 2, a:z],
                                      xT_d[:, k:k + 2, a:z])
            fetch_w2(e0, q=nc.sync)
            for i in range(1, len(nz), 2):
                a = offs[nz[i]]
                z = offs[nz[i + 1] + 1] if i + 1 < len(nz) else offs[nz[i] + 1]
                for k in range(0, KD, 2):
                    nc.sync.dma_start(xT[:, k:k + 2, a:z],
                                      xT_d[:, k:k + 2, a:z])

            def ffn1(b, n0, nt):
                h = actp.tile([128, KQ, nt], BF16, tag="h",
                              padded_shape=[128, KQ, MAX_N])
                for m in range(KQ):
                    ps = ps1.tile([128, nt], F32, tag="ps1",
                                  padded_shape=[128, MAX_N])
                    for k in range(KD):
                        nc.tensor.matmul(
                            ps[:, :],
                            w1[:, b, m, k, :],
                            xT[:, k, n0:n0 + nt],
                            start=(k == 0),
                            stop=(k == KD - 1),
                        )
                    nc.scalar.activation(h[:, m, :], ps[:, :], AF.Gelu,
                                         bias=b1[:, b, m:m + 1])
                return h

            out_q = [nc.sync, nc.gpsimd]

            def ffn2(b, n0, nt, h, ti, split_out=False):
                y = yp.tile([128, KD, nt], BF16, tag="y",
                            padded_shape=[128, KD, MAX_N])
                for md in range(KD):
                    ps = ps2.tile([128, nt], F32, tag="ps2",
                                  padded_shape=[128, MAX_N])
                    for k in range(KQ):
                        nc.tensor.matmul(
                            ps[:, :],
                            w2t[b][:, md, k, :],
                            h[:, k, :],
                            start=(k == 0),
                            stop=(k == KQ - 1),
                        )
                    nc.vector.tensor_copy(y[:, md, :], ps[:, :])
                    if split_out and md % 2 == 1:
                        out_q[(md // 2) % 2].dma_start(
                            yT_d[:, md - 1:md + 1, n0:n0 + nt],
                            y[:, md - 1:md + 1, :])
                if not split_out:
                    out_q[ti % 2].dma_start(yT_d[:, :, n0:n0 + nt],
                                            y[:, :, :])

            # Software-pipelined emission: FFN1(t) ahead of FFN2(t-1) so the
            # PE never waits on the gelu of the tile it just produced.
            # w2 blocks are prefetched two experts ahead through the ring.
            prev = None
            cur_block = None
            for ti, (b, n0, nt) in enumerate(sched):
                if b != cur_block:
                    cur_block = b
                    bi = nz.index(b)
                    if bi + 2 < len(nz):
                        fetch_w2(nz[bi + 2])
                h = ffn1(b, n0, nt)
                if prev is not None:
                    ffn2(*prev)
                prev = (b, n0, nt, h, ti)
            if prev is not None:
                # Last tile: md-pair output DMAs so the transfers hide
                # under the final matmuls instead of trailing them.
                ffn2(*prev, split_out=True)

    nc.compile()
    return nc


_PROGRAM_CACHE = {}


def _get_program(caps):
    key = tuple(caps)
    if key not in _PROGRAM_CACHE:
        _PROGRAM_CACHE[key] = build_program(caps)
    return _PROGRAM_CACHE[key]


def kernel(x, gate_w, gate_b, w1, b1, w2, b2):
    x = np.asarray(x)
    w1 = np.asarray(w1)
    b1 = np.asarray(b1)
    w2 = np.asarray(w2)
    b2 = np.asarray(b2)
    xt = x.reshape(T, D)

    # --- Routing on host (fp64; softmax is monotonic => argmax of logits) ---
    logits = xt.astype(np.float64) @ np.asarray(gate_w, np.float64)
    logits += np.asarray(gate_b, np.float64)
    eidx = np.argmax(logits, axis=-1)
    counts = np.bincount(eidx, minlength=E)

    perm = np.argsort(counts, kind="stable")
    caps = [int(-(-counts[e] // 2)) for e in perm]   # ceil(count/2)
    CT = sum(caps)
    offs = np.concatenate([[0], np.cumsum(caps)]).astype(int)

    nc = _get_program(caps)

    xt_bf = xt.astype(ml_dtypes.bfloat16)
    idxA, idxB = [], []          # per block: token indices (B may be padded)
    realB = []                   # per block: number of REAL tokens in B half
    xA = np.zeros((CT, D), ml_dtypes.bfloat16)
    xB = np.zeros((CT, D), ml_dtypes.bfloat16)
    for bI, e in enumerate(perm):
        idx = np.nonzero(eidx == e)[0]
        nA = (len(idx) + 1) // 2
        a_idx, b_idx = idx[:nA], idx[nA:]
        o = offs[bI]
        xA[o:o + len(a_idx)] = xt_bf[a_idx]
        xB[o:o + len(b_idx)] = xt_bf[b_idx]
        idxA.append(a_idx)
        idxB.append(b_idx)
        realB.append(len(b_idx))

    def to_xT(xg):
        return np.ascontiguousarray(
            xg.T.reshape(KD, 128, CT).transpose(1, 0, 2))

    xTA, xTB = to_xT(xA), to_xT(xB)

    in_maps = [None] * N_CORES
    for q in range(4):
        w1q = np.empty((128, E, KQ, KD, 128), ml_dtypes.bfloat16)
        w2q = np.empty((128, E, KD, KQ, 128), ml_dtypes.bfloat16)
        b1q = np.empty((128, E, KQ), np.float32)
        for bI, e in enumerate(perm):
            w1e = w1[e][:, q * FQ:(q + 1) * FQ]            # [D, FQ]
            w1q[:, bI] = w1e.reshape(KD, 128, KQ, 128).transpose(
                1, 2, 0, 3).astype(ml_dtypes.bfloat16)
            w2e = w2[e][q * FQ:(q + 1) * FQ, :]            # [FQ, D]
            w2q[:, bI] = w2e.reshape(KQ, 128, KD, 128).transpose(
                1, 2, 0, 3).astype(ml_dtypes.bfloat16)
            b1q[:, bI] = b1[e][q * FQ:(q + 1) * FQ].reshape(KQ, 128).T
        in_maps[q] = {"xT": xTA, "w1": w1q, "w2": w2q, "b1": b1q}
        in_maps[4 + q] = {"xT": xTB, "w1": w1q, "w2": w2q, "b1": b1q}

    res = bass_utils.run_bass_kernel_spmd(nc, in_maps,
                                          core_ids=list(range(N_CORES)),
                                          trace=TRACE)
    global LAST_RESULT
    LAST_RESULT = res

    accA = res.results[0]["yT"].astype(np.float32)
    accB = res.results[4]["yT"].astype(np.float32)
    for q in range(1, 4):
        accA += res.results[q]["yT"].astype(np.float32)
        accB += res.results[4 + q]["yT"].astype(np.float32)
    # [128, KD, CT] -> [CT, D]
    ygA = accA.transpose(1, 0, 2).reshape(D, CT).T
    ygB = accB.transpose(1, 0, 2).reshape(D, CT).T

    out = np.empty((T, D), np.float32)
    for bI, e in enumerate(perm):
        o = offs[bI]
        ia, ib, nb = idxA[bI], idxB[bI], realB[bI]
        out[ia] = ygA[o:o + len(ia)] + b2[e]
        if nb:
            out[ib] = ygB[o:o + nb] + b2[e]
    return out.reshape(B, S, D)


# revision 13
# speedup vs baseline: 1.1604x; 1.1604x over previous
"""MoE top-1 routing kernel for Trainium2 (8 NeuronCores).

Reference computation (B=8, S=1024, D=768, E=8, F=3072):
    gates = softmax(x @ gate_w + gate_b); expert_idx = argmax(gates)
    out[t] = gelu(x[t] @ w1[e] + b1[e]) @ w2[e] + b2[e]   for e = expert_idx[t]
    (no gate-probability scaling)

Strategy:
  * Routing on host in fp64 (softmax is monotonic, so argmax of logits ==
    argmax of gates; observed top-2 logit gaps are >=2e-5, far above fp32
    matmul noise, so this matches the reference's argmax).
  * Experts are split into two groups of 4, chosen at runtime to balance
    token counts.  Cores 0-3 serve group 0, cores 4-7 group 1.  Within a
    group, core q holds the q-th quarter of the F dimension of all four
    experts' weights (same SBUF footprint as one full expert) and processes
    ALL of the group's tokens, producing a partial sum of the second matmul.
    The host adds the four partials + b2.  This balances compute to within
    a few percent of T/8 tokens per core, vs ~25% padding overhead for
    straight expert-parallel dispatch.
  * Matmuls in bf16 with fp32 PSUM accumulation; activations stay
    transposed ([feature, token]) so both weight matrices act as the
    stationary matmul operand in their natural layout.  gelu (erf-based)
    on the Scalar engine with the b1 bias fused; FFN2 partial-sums are
    copied PSUM->SBUF as bf16 on the Vector engine and DMA'd out.
"""

import sys
from itertools import combinations

try:
    import concourse  # noqa: F401
except ImportError:
    sys.path.insert(0, "/opt/trn_rl_repo")

import numpy as np
import ml_dtypes

import concourse.bass as bass  # noqa: F401
import concourse.tile as tile
import concourse.mybir as mybir
from concourse import bacc
from concourse import bass_utils

BF16 = mybir.dt.bfloat16
F32 = mybir.dt.float32
AF = mybir.ActivationFunctionType

B, S, D, E = 8, 1024, 768, 8
F = 4 * D           # 3072
T = B * S           # 8192
KD = D // 128       # 6 contraction chunks over D
NQ = 4              # F-quarter factor (cores per expert group)
FQ = F // NQ        # 768 features per core
KQ = FQ // 128      # 6 chunks over the F-quarter
N_CORES = 8
MAX_N = 512         # moving-dim tile (one fp32 PSUM bank)

# Debug/profiling knobs (used by the local test harness only).
TRACE = False
LAST_RESULT = None


def _split_tiles(cap, lead=None, tail=None):
    """Split a block of `cap` tokens into ceil(cap/512) near-equal tiles.
    If `lead`/`tail` is given, the first/last tile is that size (lead kept
    small so the very first matmuls depend on only a sliver of the token
    DMA; tail kept small so the final output drain is short)."""
    if cap == 0:
        return []
    out = []
    off = 0
    tail_t = None
    if lead is not None and cap > lead:
        out.append((0, lead))
        off = lead
        cap -= lead
    if tail is not None and cap > tail + 128:
        tail_t = tail
        cap -= tail
    n = -(-cap // MAX_N)
    base, rem = divmod(cap, n)
    for i in range(n):
        sz = base + (1 if i < rem else 0)
        out.append((off, sz))
        off += sz
    if tail_t is not None:
        out.append((off, tail_t))
    return out


def build_program(caps):
    """Per-core program: 4 expert blocks with token capacities `caps`."""
    caps = list(caps)
    CT = sum(caps)
    nc = bacc.Bacc("TRN2", target_bir_lowering=False, debug=False,
                   num_devices=N_CORES)

    xT_d = nc.dram_tensor("xT", (128, KD, CT), BF16, kind="ExternalInput")
    w1_d = nc.dram_tensor("w1", (128, 4, KQ, KD, 128), BF16,
                          kind="ExternalInput")
    w2_d = nc.dram_tensor("w2", (128, 4, KD, KQ, 128), BF16,
                          kind="ExternalInput")
    b1_d = nc.dram_tensor("b1", (128, 4, KQ), F32, kind="ExternalInput")
    yT_d = nc.dram_tensor("yT", (128, KD, CT), BF16, kind="ExternalOutput")

    offs = np.concatenate([[0], np.cumsum(caps)]).astype(int)
    # Process blocks smallest-first so the first matmul's DMA dependency
    # (that block's tokens + first weight chunk) is as small as possible.
    border = sorted(range(4), key=lambda b: caps[b])
    # (block, tile-offset-within-CT, width) in execution order.  The first
    # block ramps up through small tiles so the PE can start while the bulk
    # of the token/weight DMAs are still in flight.
    nzb = [b for b in border if caps[b] > 0]
    sched = []
    for b in nzb:
        tail = 128 if b == nzb[-1] else None
        for (o, w) in _split_tiles(caps[b], tail=tail):
            sched.append((b, offs[b] + o, w))

    with tile.TileContext(nc) as tc:
        with (
            tc.tile_pool(name="wts", bufs=1) as wts,
            tc.tile_pool(name="act", bufs=2) as actp,
            tc.tile_pool(name="ps1", bufs=4, space="PSUM") as ps1,
            tc.tile_pool(name="ps2", bufs=4, space="PSUM") as ps2,
        ):
            xT = wts.tile([128, KD, CT], BF16, tag="xT")
            w1 = wts.tile([128, 4, KQ, KD, 128], BF16, tag="w1")
            w2 = wts.tile([128, 4, KD, KQ, 128], BF16, tag="w2")
            b1 = wts.tile([128, 4, KQ], F32, tag="b1")
            warm = wts.tile([128, 128], BF16, tag="warm")
            # memset on the (otherwise idle) Vector engine so GpSimd's
            # first instruction is the critical w1[b0] DMA issue.
            nc.vector.memset(warm[:], 0.0)
            wps = ps1.tile([128, 128], F32, tag="ps1",
                           padded_shape=[128, MAX_N])

            # PE warmup: dummy matmuls (~4.5us) run while the head DMAs
            # stream in, flipping the HAM clock gate to 2.4 GHz before the
            # real matmul stream starts.
            for _ in range(40):
                nc.tensor.matmul(wps[:, :], warm[:, :], warm[:, :])

            # Head DMAs in deadline order.  Sync carries the token stream
            # (first block's x per-k, then w2[b0], then later blocks' x);
            # GpSimd (otherwise idle) carries all weights: the first
            # block's w1 in per-m pieces, b1, then the later blocks'
            # w1/w2 as whole transfers.  Scalar issues NOTHING, so the
            # activation-table loads and gelus are never queued behind
            # DMA descriptor setup.
            b0 = nzb[0]
            for k in range(KD):
                nc.sync.dma_start(xT[:, k, offs[b0]:offs[b0 + 1]],
                                  xT_d[:, k, offs[b0]:offs[b0 + 1]])
            for m in range(KQ):
                nc.gpsimd.dma_start(w1[:, b0, m, :, :], w1_d[:, b0, m, :, :])
            nc.gpsimd.dma_start(b1[:], b1_d[:])
            nc.sync.dma_start(w2[:, b0, :, :, :], w2_d[:, b0, :, :, :])
            for b in nzb[1:]:
                for k in range(KD):
                    nc.sync.dma_start(xT[:, k, offs[b]:offs[b + 1]],
                                      xT_d[:, k, offs[b]:offs[b + 1]])
                nc.gpsimd.dma_start(w1[:, b, :, :, :], w1_d[:, b, :, :, :])
                nc.gpsimd.dma_start(w2[:, b, :, :, :], w2_d[:, b, :, :, :])

            def ffn1(b, n0, nt):
                h = actp.tile([128, KQ, nt], BF16, tag="h",
                              padded_shape=[128, KQ, MAX_N])
                for m in range(KQ):
                    ps = ps1.tile([128, nt], F32, tag="ps1",
                                  padded_shape=[128, MAX_N])
                    for k in range(KD):
                        nc.tensor.matmul(
                            ps[:, :],
                            w1[:, b, m, k, :],
                            xT[:, k, n0:n0 + nt],
                            start=(k == 0),
                            stop=(k == KD - 1),
                        )
                    nc.scalar.activation(h[:, m, :], ps[:, :], AF.Gelu,
                                         bias=b1[:, b, m:m + 1])
                return h

            def ffn2(b, n0, nt, h, split_out=False):
                y = actp.tile([128, KD, nt], BF16, tag="y",
                              padded_shape=[128, KD, MAX_N])
                for md in range(KD):
                    ps = ps2.tile([128, nt], F32, tag="ps2",
                                  padded_shape=[128, MAX_N])
                    for k in range(KQ):
                        nc.tensor.matmul(
                            ps[:, :],
                            w2[:, b, md, k, :],
                            h[:, k, :],
                            start=(k == 0),
                            stop=(k == KQ - 1),
                        )
                    nc.vector.tensor_copy(y[:, md, :], ps[:, :])
                    if split_out:
                        nc.sync.dma_start(yT_d[:, md, n0:n0 + nt], y[:, md, :])
                if not split_out:
                    nc.sync.dma_start(yT_d[:, :, n0:n0 + nt], y[:, :, :])

            # Software-pipelined emission: FFN1(t) ahead of FFN2(t-1) so the
            # PE never waits on the gelu of the tile it just produced.
            prev = None
            for (b, n0, nt) in sched:
                h = ffn1(b, n0, nt)
                if prev is not None:
                    ffn2(*prev)
                prev = (b, n0, nt, h)
            if prev is not None:
                # Last tile: emit the output DMA per-chunk so the transfers
                # hide under the final matmuls instead of trailing them.
                ffn2(*prev, split_out=True)

    nc.compile()
    return nc


_PROGRAM_CACHE = {}


def _get_program(caps):
    key = tuple(caps)
    if key not in _PROGRAM_CACHE:
        _PROGRAM_CACHE[key] = build_program(caps)
    return _PROGRAM_CACHE[key]


def _choose_groups(counts):
    """Partition experts into two groups of 4 minimizing sum of positionwise
    maxima of the descending-sorted counts (= padded capacity)."""
    experts = list(range(E))
    best = None
    for g0 in combinations(experts, 4):
        g1 = tuple(e for e in experts if e not in g0)
        c0 = sorted((counts[e] for e in g0), reverse=True)
        c1 = sorted((counts[e] for e in g1), reverse=True)
        caps = [max(a, b) for a, b in zip(c0, c1)]
        cost = sum(caps)
        if best is None or cost < best[0]:
            s0 = sorted(g0, key=lambda e: -counts[e])
            s1 = sorted(g1, key=lambda e: -counts[e])
            best = (cost, s0, s1, caps)
    return best[1], best[2], best[3]


def kernel(x, gate_w, gate_b, w1, b1, w2, b2):
    x = np.asarray(x)
    w1 = np.asarray(w1)
    b1 = np.asarray(b1)
    w2 = np.asarray(w2)
    b2 = np.asarray(b2)
    xt = x.reshape(T, D)

    # --- Routing on host (fp64; softmax is monotonic => argmax of logits) ---
    logits = xt.astype(np.float64) @ np.asarray(gate_w, np.float64)
    logits += np.asarray(gate_b, np.float64)
    eidx = np.argmax(logits, axis=-1)
    counts = np.bincount(eidx, minlength=E)

    groups = _choose_groups(counts)
    g_experts = [groups[0], groups[1]]
    caps = groups[2]
    CT = sum(caps)
    offs = np.concatenate([[0], np.cumsum(caps)]).astype(int)

    nc = _get_program(caps)

    xt_bf = xt.astype(ml_dtypes.bfloat16)
    tok_idx = []      # per group: token indices laid out into the CT buffer
    in_maps = [None] * N_CORES
    for g in range(2):
        idx_blocks = [np.nonzero(eidx == e)[0] for e in g_experts[g]]
        xg = np.zeros((CT, D), ml_dtypes.bfloat16)
        for b in range(4):
            o = offs[b]
            xg[o:o + len(idx_blocks[b])] = xt_bf[idx_blocks[b]]
        tok_idx.append(idx_blocks)
        # [CT, D] -> [128, KD, CT]
        xTg = np.ascontiguousarray(xg.T.reshape(KD, 128, CT).transpose(1, 0, 2))
        for q in range(NQ):
            # w1 quarter: [D, FQ] per expert -> [128, 4, KQ, KD, 128]
            w1q = np.empty((128, 4, KQ, KD, 128), ml_dtypes.bfloat16)
            w2q = np.empty((128, 4, KD, KQ, 128), ml_dtypes.bfloat16)
            b1q = np.empty((128, 4, KQ), np.float32)
            for b, e in enumerate(g_experts[g]):
                w1e = w1[e][:, q * FQ:(q + 1) * FQ]        # [D, FQ]
                w1q[:, b] = w1e.reshape(KD, 128, KQ, 128).transpose(
                    1, 2, 0, 3).astype(ml_dtypes.bfloat16)
                w2e = w2[e][q * FQ:(q + 1) * FQ, :]        # [FQ, D]
                w2q[:, b] = w2e.reshape(KQ, 128, KD, 128).transpose(
                    1, 2, 0, 3).astype(ml_dtypes.bfloat16)
                b1q[:, b] = b1[e][q * FQ:(q + 1) * FQ].reshape(KQ, 128).T
            in_maps[g * NQ + q] = {"xT": xTg, "w1": w1q, "w2": w2q, "b1": b1q}

    res = bass_utils.run_bass_kernel_spmd(nc, in_maps,
                                          core_ids=list(range(N_CORES)),
                                          trace=TRACE)
    global LAST_RESULT
    LAST_RESULT = res

    out = np.empty((T, D), np.float32)
    for g in range(2):
        acc = res.results[g * NQ][
            "yT"].astype(np.float32)
        for q in range(1, NQ):
            acc += res.results[g * NQ + q]["yT"].astype(np.float32)
        # [128, KD, CT] -> [CT, D]
        yg = acc.transpose(1, 0, 2).reshape(D, CT).T
        for b, e in enumerate(g_experts[g]):
            idx = tok_idx[g][b]
            out[idx] = yg[offs[b]:offs[b] + len(idx)] + b2[e]
    return out.reshape(B, S, D)



# revision 14
# speedup vs baseline: 1.2747x; 1.0985x over previous
"""MoE top-1 routing kernel for Trainium2 (8 NeuronCores).

Reference computation (B=8, S=1024, D=768, E=8, F=3072):
    gates = softmax(x @ gate_w + gate_b); expert_idx = argmax(gates)
    out[t] = gelu(x[t] @ w1[e] + b1[e]) @ w2[e] + b2[e]   for e = expert_idx[t]
    (no gate-probability scaling)

Strategy:
  * Routing on host in fp64 (softmax is monotonic, so argmax of logits ==
    argmax of gates; observed top-2 logit gaps are >=2e-5, far above fp32
    matmul noise, so this matches the reference's argmax).
  * Experts are split into two groups of 4, chosen at runtime to balance
    token counts.  Cores 0-3 serve group 0, cores 4-7 group 1.  Within a
    group, core q holds the q-th quarter of the F dimension of all four
    experts' weights (same SBUF footprint as one full expert) and processes
    ALL of the group's tokens, producing a partial sum of the second matmul.
    The host adds the four partials + b2.  This balances compute to within
    a few percent of T/8 tokens per core, vs ~25% padding overhead for
    straight expert-parallel dispatch.
  * Matmuls in bf16 with fp32 PSUM accumulation; activations stay
    transposed ([feature, token]) so both weight matrices act as the
    stationary matmul operand in their natural layout.  gelu (erf-based)
    on the Scalar engine with the b1 bias fused; FFN2 partial-sums are
    copied PSUM->SBUF as bf16 on the Vector engine and DMA'd out.
"""

import sys
from itertools import combinations

try:
    import concourse  # noqa: F401
except ImportError:
    sys.path.insert(0, "/opt/trn_rl_repo")

import numpy as np
import ml_dtypes

import concourse.bass as bass  # noqa: F401
import concourse.tile as tile
import concourse.mybir as mybir
from concourse import bacc
from concourse import bass_utils

BF16 = mybir.dt.bfloat16
F32 = mybir.dt.float32
AF = mybir.ActivationFunctionType

B, S, D, E = 8, 1024, 768, 8
F = 4 * D           # 3072
T = B * S           # 8192
KD = D // 128       # 6 contraction chunks over D
NQ = 4              # F-quarter factor (cores per expert group)
FQ = F // NQ        # 768 features per core
KQ = FQ // 128      # 6 chunks over the F-quarter
N_CORES = 8
MAX_N = 512         # moving-dim tile (one fp32 PSUM bank)

# Debug/profiling knobs (used by the local test harness only).
TRACE = False
LAST_RESULT = None


def _split_tiles(cap, lead=None):
    """Split a block of `cap` tokens into ceil(cap/512) near-equal tiles.
    If `lead` is given, the first tile is that size (kept small so the very
    first matmuls depend on only a sliver of the token DMA)."""
    if cap == 0:
        return []
    out = []
    off = 0
    if lead is not None and cap > lead:
        out.append((0, lead))
        off = lead
        cap -= lead
    n = -(-cap // MAX_N)
    base, rem = divmod(cap, n)
    for i in range(n):
        sz = base + (1 if i < rem else 0)
        out.append((off, sz))
        off += sz
    return out


def build_program(caps):
    """Per-core program: 4 expert blocks with token capacities `caps`."""
    caps = list(caps)
    CT = sum(caps)
    nc = bacc.Bacc("TRN2", target_bir_lowering=False, debug=False,
                   num_devices=N_CORES)

    xT_d = nc.dram_tensor("xT", (128, KD, CT), BF16, kind="ExternalInput")
    w1_d = nc.dram_tensor("w1", (128, 4, KQ, KD, 128), BF16,
                          kind="ExternalInput")
    w2_d = nc.dram_tensor("w2", (128, 4, KD, KQ, 128), BF16,
                          kind="ExternalInput")
    b1_d = nc.dram_tensor("b1", (128, 4, KQ), F32, kind="ExternalInput")
    yT_d = nc.dram_tensor("yT", (128, KD, CT), BF16, kind="ExternalOutput")

    offs = np.concatenate([[0], np.cumsum(caps)]).astype(int)
    # Process blocks smallest-first so the first matmul's DMA dependency
    # (that block's tokens + first weight chunk) is as small as possible.
    border = sorted(range(4), key=lambda b: caps[b])
    # (block, tile-offset-within-CT, width) in execution order.  The first
    # block ramps up through small tiles so the PE can start while the bulk
    # of the token/weight DMAs are still in flight.
    sched = []
    for b in border:
        for (o, w) in _split_tiles(caps[b]):
            sched.append((b, offs[b] + o, w))

    with tile.TileContext(nc) as tc:
        with (
            tc.tile_pool(name="wts", bufs=1) as wts,
            tc.tile_pool(name="act", bufs=2) as actp,
            tc.tile_pool(name="ps1", bufs=4, space="PSUM") as ps1,
            tc.tile_pool(name="ps2", bufs=4, space="PSUM") as ps2,
        ):
            xT = wts.tile([128, KD, CT], BF16, tag="xT")
            w1 = wts.tile([128, 4, KQ, KD, 128], BF16, tag="w1")
            w2 = wts.tile([128, 4, KD, KQ, 128], BF16, tag="w2")
            b1 = wts.tile([128, 4, KQ], F32, tag="b1")
            warm = wts.tile([128, 128], BF16, tag="warm")
            nc.gpsimd.memset(warm[:], 0.0)
            wps = ps1.tile([128, 128], F32, tag="ps1",
                           padded_shape=[128, MAX_N])

            # PE warmup: dummy matmuls (~4.5us) run while the head DMAs
            # stream in, flipping the HAM clock gate to 2.4 GHz before the
            # real matmul stream starts.
            for _ in range(40):
                nc.tensor.matmul(wps[:, :], warm[:, :], warm[:, :])

            # Head DMAs: the first (small) tile's dependencies, issued
            # round-robin across three otherwise-idle engine queues so the
            # ~650ns per-issue cost doesn't serialize; then the bulk loads
            # on Sync (their issue cost hides under compute).
            b0 = border[0]
            for k in range(KD):
                nc.sync.dma_start(xT[:, k, offs[b0]:offs[b0 + 1]],
                                  xT_d[:, k, offs[b0]:offs[b0 + 1]])
            for m in range(KQ):
                nc.scalar.dma_start(w1[:, b0, m, :, :], w1_d[:, b0, m, :, :])
            nc.scalar.dma_start(b1[:], b1_d[:])
            nc.sync.dma_start(w2[:, b0, :, :, :], w2_d[:, b0, :, :, :])
            for b in border[1:]:
                if caps[b] == 0:
                    continue
                for k in range(KD):
                    nc.sync.dma_start(xT[:, k, offs[b]:offs[b + 1]],
                                      xT_d[:, k, offs[b]:offs[b + 1]])
                nc.sync.dma_start(w1[:, b, :, :, :], w1_d[:, b, :, :, :])
                nc.sync.dma_start(w2[:, b, :, :, :], w2_d[:, b, :, :, :])

            def ffn1(b, n0, nt):
                h = actp.tile([128, KQ, nt], BF16, tag="h",
                              padded_shape=[128, KQ, MAX_N])
                for m in range(KQ):
                    ps = ps1.tile([128, nt], F32, tag="ps1",
                                  padded_shape=[128, MAX_N])
                    for k in range(KD):
                        nc.tensor.matmul(
                            ps[:, :],
                            w1[:, b, m, k, :],
                            xT[:, k, n0:n0 + nt],
                            start=(k == 0),
                            stop=(k == KD - 1),
                        )
                    nc.scalar.activation(h[:, m, :], ps[:, :], AF.Gelu,
                                         bias=b1[:, b, m:m + 1])
                return h

            def ffn2(b, n0, nt, h, split_out=False):
                y = actp.tile([128, KD, nt], BF16, tag="y",
                              padded_shape=[128, KD, MAX_N])
                for md in range(KD):
                    ps = ps2.tile([128, nt], F32, tag="ps2",
                                  padded_shape=[128, MAX_N])
                    for k in range(KQ):
                        nc.tensor.matmul(
                            ps[:, :],
                            w2[:, b, md, k, :],
                            h[:, k, :],
                            start=(k == 0),
                            stop=(k == KQ - 1),
                        )
                    nc.vector.tensor_copy(y[:, md, :], ps[:, :])
                    if split_out:
                        nc.sync.dma_start(yT_d[:, md, n0:n0 + nt], y[:, md, :])
                if not split_out:
                    nc.sync.dma_start(yT_d[:, :, n0:n0 + nt], y[:, :, :])

            # Software-pipelined emission: FFN1(t) ahead of FFN2(t-1) so the
            # PE never waits on the gelu of the tile it just produced.
            prev = None
            for (b, n0, nt) in sched:
                h = ffn1(b, n0, nt)
                if prev is not None:
                    ffn2(*prev)
                prev = (b, n0, nt, h)
            if prev is not None:
                # Last tile: emit the output DMA per-chunk so the transfers
                # hide under the final matmuls instead of trailing them.
                ffn2(*prev, split_out=True)

    nc.compile()
    return nc


_PROGRAM_CACHE = {}


def _get_program(caps):
    key = tuple(caps)
    if key not in _PROGRAM_CACHE:
        _PROGRAM_CACHE[key] = build_program(caps)
    return _PROGRAM_CACHE[key]


def _choose_groups(counts):
    """Partition experts into two groups of 4 minimizing sum of positionwise
    maxima of the descending-sorted counts (= padded capacity)."""
    experts = list(range(E))
    best = None
    for g0 in combinations(experts, 4):
        g1 = tuple(e for e in experts if e not in g0)
        c0 = sorted((counts[e] for e in g0), reverse=True)
        c1 = sorted((counts[e] for e in g1), reverse=True)
        caps = [max(a, b) for a, b in zip(c0, c1)]
        cost = sum(caps)
        if best is None or cost < best[0]:
            s0 = sorted(g0, key=lambda e: -counts[e])
            s1 = sorted(g1, key=lambda e: -counts[e])
            best = (cost, s0, s1, caps)
    return best[1], best[2], best[3]


def kernel(x, gate_w, gate_b, w1, b1, w2, b2):
    x = np.asarray(x)
    w1 = np.asarray(w1)
    b1 = np.asarray(b1)
    w2 = np.asarray(w2)
    b2 = np.asarray(b2)
    xt = x.reshape(T, D)

    # --- Routing on host (fp64; softmax is monotonic => argmax of logits) ---
    logits = xt.astype(np.float64) @ np.asarray(gate_w, np.float64)
    logits += np.asarray(gate_b, np.float64)
    eidx = np.argmax(logits, axis=-1)
    counts = np.bincount(eidx, minlength=E)

    groups = _choose_groups(counts)
    g_experts = [groups[0], groups[1]]
    caps = groups[2]
    CT = sum(caps)
    offs = np.concatenate([[0], np.cumsum(caps)]).astype(int)

    nc = _get_program(caps)

    xt_bf = xt.astype(ml_dtypes.bfloat16)
    tok_idx = []      # per group: token indices laid out into the CT buffer
    in_maps = [None] * N_CORES
    for g in range(2):
        idx_blocks = [np.nonzero(eidx == e)[0] for e in g_experts[g]]
        xg = np.zeros((CT, D), ml_dtypes.bfloat16)
        for b in range(4):
            o = offs[b]
            xg[o:o + len(idx_blocks[b])] = xt_bf[idx_blocks[b]]
        tok_idx.append(idx_blocks)
        # [CT, D] -> [128, KD, CT]
        xTg = np.ascontiguousarray(xg.T.reshape(KD, 128, CT).transpose(1, 0, 2))
        for q in range(NQ):
            # w1 quarter: [D, FQ] per expert -> [128, 4, KQ, KD, 128]
            w1q = np.empty((128, 4, KQ, KD, 128), ml_dtypes.bfloat16)
            w2q = np.empty((128, 4, KD, KQ, 128), ml_dtypes.bfloat16)
            b1q = np.empty((128, 4, KQ), np.float32)
            for b, e in enumerate(g_experts[g]):
                w1e = w1[e][:, q * FQ:(q + 1) * FQ]        # [D, FQ]
                w1q[:, b] = w1e.reshape(KD, 128, KQ, 128).transpose(
                    1, 2, 0, 3).astype(ml_dtypes.bfloat16)
                w2e = w2[e][q * FQ:(q + 1) * FQ, :]        # [FQ, D]
                w2q[:, b] = w2e.reshape(KQ, 128, KD, 128).transpose(
                    1, 2, 0, 3).astype(ml_dtypes.bfloat16)
                b1q[:, b] = b1[e][q * FQ:(q + 1) * FQ].reshape(KQ, 128).T
            in_maps[g * NQ + q] = {"xT": xTg, "w1": w1q, "w2": w2q, "b1": b1q}

    res = bass_utils.run_bass_kernel_spmd(nc, in_maps,
                                          core_ids=list(range(N_CORES)),
                                          trace=TRACE)
    global LAST_RESULT
    LAST_RESULT = res

    out = np.empty((T, D), np.float32)
    for g in range(2):
        acc = res.results[g * NQ][
            "yT"].astype(np.float32)
        for q in range(1, NQ):
            acc += res.results[g * NQ + q]["yT"].astype(np.float32)
        # [128, KD, CT] -> [CT, D]
        yg = acc.transpose(1, 0, 2).reshape(D, CT).T
        for b, e in enumerate(g_experts[g]):
            idx = tok_idx[g][b]
            out[idx] = yg[offs[b]:offs[b] + len(idx)] + b2[e]
    return out.reshape(B, S, D)

